# revision 8
# baseline (speedup 1.0000x reference)
"""Binarized dense layer (tanh(sign(x) @ sign(w) + b)) on 8 Trainium2 cores.

Full input shapes (hardcoded): inputs [8192, 4096] f32, kernel [4096, 4096] f32,
bias [4096] f32 -> out [8192, 4096] f32.

Sharding: 4 batch shards x 2 output-column shards (core i -> r=i//2, c=i%2).
Per core: x [2048, 4096], w [4096, 2048], b [2048] -> y [2048, 2048].

Per-core kernel (Tile framework):
  - W: DMA f32 row-chunks, binarize to +-1 fp8e4 on ACT (Sign), kept resident
    in SBUF laid out [128, 2, O] per 256-row K-pair for DoubleRow matmul.
  - X: DMA f32 m-tiles [128, 4096], transpose 128x128 blocks on PE (f32, via
    identity) into PSUM, binarize to +-0.5 fp8e4 on DVE while evicting
    PSUM->SBUF (tensor_scalar is_ge 0.0 then subtract 0.5).
  - Matmul: fp8 DoubleRow, K=256 per step, N=512 (one PSUM bank), M=128.
    psum accumulates 0.5 * (+-1 dot) exactly in f32.
  - Output: tanh on ACT reading PSUM with scale=2.0, staged [128, O] in SBUF,
    one 1MB DMA per m-tile.
"""

import sys
import types

if "/opt/trn_rl_repo" not in sys.path:
    sys.path.insert(0, "/opt/trn_rl_repo")

from contextlib import ExitStack

import numpy as np


def _ensure_ntff_hook_module():
    """The RL image's antenv lacks axon_hooks, which bass_utils imports for
    trace=True under axon. Register a functional shim in sys.modules."""
    name = "antenv.axon_hooks"
    if name in sys.modules:
        return
    try:
        import antenv
        __import__(name)
        return  # real module exists
    except ImportError:
        pass
    mod = types.ModuleType(name)
    mod._hook = None

    def set_axon_ntff_profile_hook(hook):
        mod._hook = hook

    def get_axon_ntff_profile_hook():
        if mod._hook is None:
            try:
                from trn_agent_boot.trn_boot import _ntff_profile_via_ctypes
                mod._hook = _ntff_profile_via_ctypes("/opt/axon/libaxon_pjrt.so")
            except Exception:
                return None
        return mod._hook

    mod.set_axon_ntff_profile_hook = set_axon_ntff_profile_hook
    mod.get_axon_ntff_profile_hook = get_axon_ntff_profile_hook
    sys.modules[name] = mod
    try:
        import antenv
        antenv.axon_hooks = mod
    except ImportError:
        pass


_ensure_ntff_hook_module()

import concourse.bass as bass
import concourse.tile as tile
from concourse import bacc, mybir
from concourse.bass_utils import run_bass_kernel_spmd
from concourse.masks import make_identity

F32 = mybir.dt.float32
FP8 = mybir.dt.float8e4

N_CORES = 8
R_SHARDS = 4  # batch shards
C_SHARDS = 2  # output-column shards

B_FULL, D_FULL, O_FULL = 8192, 4096, 4096
B_LOC = B_FULL // R_SHARDS   # 2048
O_LOC = O_FULL // C_SHARDS   # 2048


def build_nc(b_loc=B_LOC, d=D_FULL, o_loc=O_LOC, bias_nonzero=False,
             w_gather=True):
    """Build the per-core Bass program (identical across cores).

    With w_gather=True the "w" input is only this core's quarter of the
    W column-slice (rows [r*d/4, (r+1)*d/4)); the 4 batch-shard cores
    sharing a column half binarize their quarters to fp8 and AllGather
    them over on-chip links, cutting W HBM traffic 4x.
    """
    assert b_loc % 128 == 0 and d % 256 == 0 and o_loc % 512 == 0
    M = b_loc // 128    # m-tiles (output rows per 128-partition tile)
    KP = d // 256       # DoubleRow K-pairs
    KC = d // 128       # 128-row chunks of the contraction dim
    N = o_loc // 512    # n-tiles (one PSUM bank each)
    DQ = d // R_SHARDS  # W rows per core before gather

    nc = bacc.Bacc("TRN2", target_bir_lowering=False, debug=False,
                   num_devices=N_CORES)
    w_rows = DQ if w_gather else d
    x = nc.dram_tensor("x", [b_loc, d], F32, kind="ExternalInput")
    w = nc.dram_tensor("w", [w_rows, o_loc], F32, kind="ExternalInput")
    b = nc.dram_tensor("b", [o_loc], F32, kind="ExternalInput")
    y = nc.dram_tensor("y", [b_loc, o_loc], F32, kind="ExternalOutput")

    with tile.TileContext(nc) as tc, ExitStack() as ctx:
        singles = ctx.enter_context(tc.tile_pool(name="singles", bufs=1))
        wstage = ctx.enter_context(tc.tile_pool(name="wstage", bufs=3))
        wbp = ctx.enter_context(tc.tile_pool(name="wb", bufs=KP))
        xstage = ctx.enter_context(tc.tile_pool(name="xs", bufs=3))
        xbtp = ctx.enter_context(tc.tile_pool(name="xbt", bufs=2))
        ostage = ctx.enter_context(tc.tile_pool(name="ost", bufs=3))
        pstage = ctx.enter_context(tc.tile_pool(name="pt", bufs=2, space="PSUM"))
        pacc = ctx.enter_context(tc.tile_pool(name="pa", bufs=6, space="PSUM"))

        ident = singles.tile([128, 128], F32)
        make_identity(nc, ident)

        bias_bc = None
        if bias_nonzero:
            bias_bc = singles.tile([128, o_loc], F32)
            bias_ap = bass.AP(tensor=b.ap().tensor, offset=0,
                              ap=[[0, 128], [1, o_loc]])
            nc.gpsimd.dma_start(out=bias_bc[:], in_=bias_ap)
            # psum holds S/2 and tanh applies scale=2.0, so add bias/2
            nc.vector.tensor_scalar_mul(bias_bc[:], bias_bc[:], 0.5)

        # ---- W preload: binarize to +-1 fp8, resident in SBUF ----
        # wb[k] is [128, 2, o_loc]: (p, j, n) = sign(w[k*256 + j*128 + p, n]).
        # W DMAs go on the sync queue; X DMAs go on the scalar queue so early
        # X tiles are not serialized behind the whole W preload.
        wb = []
        if w_gather:
            dramp = ctx.enter_context(tc.tile_pool(name="dram", bufs=1,
                                                   space="DRAM"))
            w8stage = ctx.enter_context(tc.tile_pool(name="w8s", bufs=3))
            wq = dramp.tile([DQ, o_loc], FP8)
            wg = dramp.tile([d, o_loc], FP8)
            for q in range(DQ // 128):
                s = wstage.tile([128, o_loc], F32, tag="ws")
                nc.sync.dma_start(out=s[:], in_=w[q * 128:(q + 1) * 128, :])
                s8 = w8stage.tile([128, o_loc], FP8, tag="w8")
                nc.scalar.activation(out=s8[:], in_=s[:],
                                     func=mybir.ActivationFunctionType.Sign)
                nc.sync.dma_start(out=wq[q * 128:(q + 1) * 128, :], in_=s8[:])
            groups = [[2 * r + c for r in range(R_SHARDS)]
                      for c in range(C_SHARDS)]
            nc.gpsimd.collective_compute(
                "AllGather", mybir.AluOpType.bypass,
                replica_groups=groups, ins=[wq[:]], outs=[wg[:]])
            for k in range(KP):
                t = wbp.tile([128, 2, o_loc], FP8, tag="wb")
                nc.sync.dma_start(
                    out=t[:], in_=wg[2 * k * 128:(2 * k + 2) * 128, :]
                    .rearrange("(j p) n -> p j n", j=2))
                wb.append(t)
        else:
            for k in range(KP):
                t = wbp.tile([128, 2, o_loc], FP8, tag="wb")
                for j in (0, 1):
                    s = wstage.tile([128, o_loc], F32, tag="ws")
                    nc.sync.dma_start(out=s[:], in_=w[(2 * k + j) * 128:(2 * k + j + 1) * 128, :])
                    nc.scalar.activation(out=t[:, j, :], in_=s[:],
                                         func=mybir.ActivationFunctionType.Sign)
                wb.append(t)

        # ---- main loop over m-tiles ----
        for m in range(M):
            xs = xstage.tile([128, d], F32, tag="xs")
            nc.scalar.dma_start(out=xs[:], in_=x[m * 128:(m + 1) * 128, :])

            # transpose + binarize: xbt[p, c, mm] = +-0.5 of x[m*128+mm, c*128+p]
            xbt = xbtp.tile([128, KC, 128], FP8, tag="xbt")
            for q in range(KC // 4):
                pt = pstage.tile([128, 512], F32, tag="pt")
                for i in range(4):
                    c = 4 * q + i
                    nc.tensor.transpose(pt[:, i * 128:(i + 1) * 128],
                                        xs[:, c * 128:(c + 1) * 128], ident[:])
                nc.vector.tensor_scalar(
                    out=xbt[:, 4 * q:4 * q + 4, :], in0=pt[:],
                    scalar1=0.0, scalar2=0.5,
                    op0=mybir.AluOpType.is_ge, op1=mybir.AluOpType.subtract)

            pa = [pacc.tile([128, 512], F32, tag="pa", name=f"pa_{m}_{n}")
                  for n in range(N)]
            for k in range(KP):
                lhsT = xbt[:, 2 * k:2 * k + 2, :]
                for n in range(N):
                    nc.tensor.matmul(
                        pa[n][:], lhsT, wb[k][:, :, n * 512:(n + 1) * 512],
                        start=(k == 0), stop=(k == KP - 1),
                        perf_mode=mybir.MatmulPerfMode.DoubleRow)

            o = ostage.tile([128, o_loc], F32, tag="o")
            for n in range(N):
                pn = pa[n][:]
                if bias_bc is not None:
                    nc.vector.tensor_tensor(
                        out=pn, in0=pn, in1=bias_bc[:, n * 512:(n + 1) * 512],
                        op=mybir.AluOpType.add)
                nc.scalar.activation(out=o[:, n * 512:(n + 1) * 512], in_=pn,
                                     func=mybir.ActivationFunctionType.Tanh,
                                     scale=2.0)
            nc.sync.dma_start(out=y[m * 128:(m + 1) * 128, :], in_=o[:])

    nc.compile()
    return nc


_NC_CACHE = {}


def _get_nc(key, **kwargs):
    if key not in _NC_CACHE:
        _NC_CACHE[key] = build_nc(**kwargs)
    return _NC_CACHE[key]


def kernel(inputs: np.ndarray, kernel: np.ndarray, bias: np.ndarray,
           _trace: bool = False, _trace_cores=None) -> np.ndarray:
    x = np.ascontiguousarray(inputs, dtype=np.float32)
    w = np.ascontiguousarray(kernel, dtype=np.float32)
    b = np.ascontiguousarray(bias, dtype=np.float32)
    assert x.shape == (B_FULL, D_FULL) and w.shape == (D_FULL, O_FULL)

    bias_nonzero = bool(np.any(b != 0))
    nc = _get_nc(("full", bias_nonzero), bias_nonzero=bias_nonzero)

    dq = D_FULL // R_SHARDS
    in_maps = []
    for i in range(N_CORES):
        r, c = i // C_SHARDS, i % C_SHARDS
        in_maps.append({
            "x": x[r * B_LOC:(r + 1) * B_LOC, :],
            "w": np.ascontiguousarray(
                w[r * dq:(r + 1) * dq, c * O_LOC:(c + 1) * O_LOC]),
            "b": np.ascontiguousarray(b[c * O_LOC:(c + 1) * O_LOC]),
        })

    res = run_bass_kernel_spmd(nc, in_maps, list(range(N_CORES)),
                               trace=_trace, trace_cores=_trace_cores)

    out = np.empty((B_FULL, O_FULL), dtype=np.float32)
    for i in range(N_CORES):
        r, c = i // C_SHARDS, i % C_SHARDS
        out[r * B_LOC:(r + 1) * B_LOC, c * O_LOC:(c + 1) * O_LOC] = \
            res.results[i]["y"]

    if _trace:
        return out, res
    return out
